# revision 12
# baseline (speedup 1.0000x reference)
"""Causal self-attention (B=4, S=2048, D=1024, single head) on 8 TRN2 cores.

Sharding: data-parallel over batch (4 batches x 2 cores). The two cores of a
batch split the 16 query tiles (128 rows each): core A takes tiles
{0,2,4,6,9,11,13,15}, core B the complement — slot s (s=0..7) of either core
attends to strips j=0..2s+1 (2s+2 key tiles), so both cores do exactly 72
key-tile units of causal work under ONE shared SPMD program (the <=1 waste
tile per slot is killed by the mask input).

v2 dataflow: the Q and K projections are fused on the host into
W_qk = Wq @ Wk^T (weights-only preprocessing), so the device computes
Q'' = x_q @ W_qk in ONE projection and scores = Q'' X^T directly.  Scores are
computed TRANSPOSED, key-tile-outer: for key tile j the PE produces
scoresT[j] = (X^T strip j)^T-stationary vs Q''^T-moving = [128 keys x all
queries of slots >= j//2].  The exp'd strips (pbT) are therefore already
key-major — the P^T transposes of v1 (16 xbar DMAs + PE bridging) vanish
entirely.  The causal mask is a single [128,128] DVE add on the first query
block of each strip (host-built table, 0 / -1920 upper-tri at raw-score
scale; exp applies the 1/32 softmax scale via the ACT scale field).

Per slot s: R^T = P X accumulated over its 2s+2 pbT strips (lhsT = pbT
block, rhs = key-major X), with the softmax row-sum riding along as an N=1
ones-matmul on the same stationary operand into a per-slot PSUM column;
R^T is normalized by 1/rowsum during the PSUM->SBUF copy (so no final
rescale), transposed to d-major via one xbar DMA, and multiplied by Wv.
Strips and slot tails are software-pipelined (tail_a lags the enabling
strip by one, tail_b by two) so the serial handoffs (exp -> R, rp -> rt ->
r2 DMA -> Wv apply) are covered by neighbouring PE work.

All matmuls bf16 (fp8 DoubleRow was simulated end-to-end and fails the 2e-2
gate: each fp8 operand alone contributes ~2.5-3e-2 because sharp-attention
rows carry ~10x magnitude).  PE work/core: 1.07G (Q'' apply) + 1.21G
(scoresT) + 1.21G (R^T) + 1.07G (Wv) = 4.56G MAC ~= 121us @ 2.4GHz.
"""

import os
from contextlib import ExitStack

import ml_dtypes
import numpy as np

import concourse.bacc as bacc
import concourse.mybir as mybir
import concourse.tile as tile
from concourse.bass_utils import run_bass_kernel_spmd

B, S, D = 4, 2048, 1024
P = 128
DC = D // P  # 8 contraction chunks
NT = S // P  # 16 key tiles
A_TILES = [0, 2, 4, 6, 9, 11, 13, 15]
B_TILES = [1, 3, 5, 7, 8, 10, 12, 14]
MASK_RAW = -1920.0  # -60 at logit scale, applied pre-ACT at raw score scale

F32 = mybir.dt.float32
BF16 = mybir.dt.bfloat16

_compiled = {}


def _build_v2():
    nc = bacc.Bacc("TRN2", target_bir_lowering=False, debug=False)
    # all inputs partition-major ([128, ...] with one contiguous run per
    # partition) so every DMA trigger is ~128 descriptors (~0.7us issue)
    xqh0 = nc.dram_tensor("xqh0", [P, DC, 512], BF16, kind="ExternalInput").ap()
    xqh1 = nc.dram_tensor("xqh1", [P, DC, 512], BF16, kind="ExternalInput").ap()
    xk = nc.dram_tensor("xk", [P, NT, D], BF16, kind="ExternalInput").ap()
    wqk = nc.dram_tensor("wqk", [P, DC, D], BF16, kind="ExternalInput").ap()
    wv = nc.dram_tensor("wv", [P, DC, D], BF16, kind="ExternalInput").ap()
    mcf = nc.dram_tensor("mcf", [P, 32], F32, kind="ExternalInput").ap()
    out_d = nc.dram_tensor("out", [1024, D], F32, kind="ExternalOutput").ap()

    with tile.TileContext(nc) as tc:
        _body_v2(tc, xqh0, xqh1, xk, wqk, wv, mcf, out_d)
    nc.compile()
    return nc


def _body_v2(tc, xqh0, xqh1, xk, wqk, wv, mcf, out_d):
    nc = tc.nc
    with ExitStack() as top:
        const_pool = top.enter_context(tc.tile_pool(name="cst", bufs=1))
        ones_bf = const_pool.tile([P, 1], BF16, name="ones_bf", tag="ones")
        nc.gpsimd.memset(ones_bf[:], 1.0)
        # upper-triangular mask const: TRI[p,c] = -1920 where key p > query c
        tri = const_pool.tile([P, P], F32, name="tri", tag="tri")
        nc.gpsimd.memset(tri[:], MASK_RAW)
        nc.gpsimd.affine_select(
            out=tri[:],
            in_=tri[:],
            compare_op=mybir.AluOpType.is_gt,
            fill=0.0,
            base=0,
            pattern=[[-1, P]],
            channel_multiplier=1,
        )
        mcf_t = const_pool.tile([P, 32], F32, name="mcf_t", tag="mcf")

        # whole-kernel residents (single big tiles -> few DMA triggers; the
        # DMA trigger queue issues one DIRECT2D per ~600ns, so 72 separate
        # dma_starts would gate the strip phase by ~9us)
        res_pool = top.enter_context(tc.tile_pool(name="res", bufs=1))
        # per-dc XT tiles and per-quarter XK tiles: separate tiles give
        # separate DMA-completion deps, so consumers start in arrival order
        XK_g = [
            res_pool.tile([P, 4, D], BF16, name=f"xkg{g}", tag=f"xkg{g}") for g in range(4)
        ]
        # d-major score strips, produced on-device from XK via xbar
        # block-transposes (x ships ONCE): XT_alt[j][pd,dc,k] = x[j*128+k, dc*128+pd]
        XT_alt = [
            res_pool.tile([P, DC, P], BF16, name=f"xta{j}", tag=f"xta{j}") for j in range(NT)
        ]
        Q2T = [res_pool.tile([P, 1024], BF16, name=f"q2t{d}", tag=f"q2t{d}") for d in range(DC)]
        wv_all = res_pool.tile([P, DC, D], BF16, name="wv_all", tag="wv_all")
        pbT = [res_pool.tile([P, 1024], BF16, name=f"pbt{j}", tag=f"pbt{j}") for j in range(NT)]
        msk_t = [res_pool.tile([P, P], BF16, name=f"msk{j}", tag=f"msk{j}") for j in range(NT)]

        # ---------------- Q'' projection (Q''^T = W_qk^T xq^T) ----------------
        with ExitStack() as ph:
            wqk_pool = ph.enter_context(tc.tile_pool(name="wqkp", bufs=1))
            xq_pool = ph.enter_context(tc.tile_pool(name="xqp", bufs=1))
            pps = ph.enter_context(tc.tile_pool(name="pps", bufs=1, space="PSUM"))

            wqk_all = wqk_pool.tile([P, DC, D], BF16, name="wqk_all", tag="wqk_all")
            xq_h = [
                xq_pool.tile([P, DC, 512], BF16, name=f"xq_h{h}", tag=f"xq_h{h}")
                for h in range(2)
            ]

            # DMA issue order = priority, matched to consumer deadlines:
            # phase-1 strip pairs paced per-e, then xq half 1 (qc=1 sweeps),
            # XT+first masks (strip 0 at ~40us), XK in j-quarters (tail_a
            # deadlines), wv (first tail_b ~58us), rest.
            nc.sync.dma_start(mcf_t[:], mcf[:, :])
            for e in range(DC):
                nc.sync.dma_start(wqk_all[:, e, :], wqk[:, e, :])
                nc.sync.dma_start(xq_h[0][:, e, :], xqh0[:, e, :])
            nc.sync.dma_start(xq_h[1][:, 0:4, :], xqh1[:, 0:4, :])
            nc.sync.dma_start(xq_h[1][:, 4:DC, :], xqh1[:, 4:DC, :])
            nc.sync.dma_start(XK_g[0][:, :, :], xk[:, 0:4, :])
            nc.sync.dma_start(XK_g[1][:, :, :], xk[:, 4:8, :])
            nc.sync.dma_start(wv_all[:, :, :], wv[:, :, :])
            nc.sync.dma_start(XK_g[2][:, :, :], xk[:, 8:12, :])
            nc.sync.dma_start(XK_g[3][:, :, :], xk[:, 12:NT, :])
            # first half of the score-strip transposes; the rest are
            # interleaved into the strip loop to stay behind their xk data
            # without head-blocking the r2 transposes
            for j in range(8):
                nc.sync.dma_start_transpose(
                    XT_alt[j][:, :, :], XK_g[j // 4][:, j % 4, :]
                )
            # per-strip masks built on-device: msk_t[j] = TRI*av_j + s2_j
            for j in range(NT):
                nc.vector.tensor_scalar(
                    msk_t[j][:],
                    tri[:],
                    mcf_t[:, j : j + 1],
                    mcf_t[:, NT + j : NT + j + 1],
                    op0=mybir.AluOpType.mult,
                    op1=mybir.AluOpType.add,
                )

            # e-outer accumulation across 8 live PSUM banks: the e=0 sweep
            # starts as soon as the first strip pair lands.
            for qc in range(2):
                qps = [pps.tile([P, 512], F32, name="pps", tag=f"pps{dc}") for dc in range(DC)]
                for e in range(DC):
                    for dc in range(DC):
                        nc.tensor.matmul(
                            qps[dc][:],
                            lhsT=wqk_all[:, e, dc * P : (dc + 1) * P],
                            rhs=xq_h[qc][:, e, :],
                            start=(e == 0),
                            stop=(e == DC - 1),
                        )
                for dc in range(DC):
                    nc.scalar.copy(Q2T[dc][:, qc * 512 : (qc + 1) * 512], qps[dc][:])

        # ---------------- attention: key-tile-outer strips + slot tails ----
        with ExitStack() as ph:
            rt_pool = ph.enter_context(tc.tile_pool(name="rtp", bufs=2))
            r2_pool = ph.enter_context(tc.tile_pool(name="r2p", bufs=2))
            o_pool = ph.enter_context(tc.tile_pool(name="op", bufs=2))
            small_pool = ph.enter_context(tc.tile_pool(name="smallp", bufs=3))
            qk_ps = ph.enter_context(tc.tile_pool(name="qkps", bufs=3, space="PSUM"))
            rp_ps = ph.enter_context(tc.tile_pool(name="rps", bufs=1, space="PSUM"))
            o_ps = ph.enter_context(tc.tile_pool(name="ops", bufs=2, space="PSUM"))
            ls_ps = ph.enter_context(tc.tile_pool(name="lsps", bufs=1, space="PSUM"))

            lsums = ls_ps.tile([P, 8], F32, name="lsums", tag="lsums")
            state = {}

            def strip(j):
                qlo = (j // 2) * P
                w = 1024 - qlo
                for c0 in range(qlo, 1024, 512):
                    cw = min(512, 1024 - c0)
                    ps = qk_ps.tile([P, 512], F32, name="qk_t", tag="qk_t")
                    for dc in range(DC):
                        nc.tensor.matmul(
                            ps[:, :cw],
                            lhsT=XT_alt[j][:, dc, :],
                            rhs=Q2T[dc][:, c0 : c0 + cw],
                            start=(dc == 0),
                            stop=(dc == DC - 1),
                        )
                    if c0 == qlo:
                        # causal mask lands only on the strip's first query
                        # block (host-built: 0 / -1920 / upper-tri)
                        nc.vector.tensor_add(ps[:, 0:P], ps[:, 0:P], msk_t[j][:])
                    nc.scalar.activation(
                        pbT[j][:, c0 : c0 + cw],
                        ps[:, :cw],
                        mybir.ActivationFunctionType.Exp,
                        scale=1.0 / 32.0,
                    )

            def tail_a(s):
                n = 2 * s + 2  # strips for this slot
                rp = rp_ps.tile([P, 1024], F32, name="rp", tag="rp")
                for j in range(n):
                    blk = pbT[j][:, s * P : (s + 1) * P]
                    for h in range(2):
                        nc.tensor.matmul(
                            rp[:, h * 512 : (h + 1) * 512],
                            lhsT=blk,
                            rhs=XK_g[j // 4][:, j % 4, h * 512 : (h + 1) * 512],
                            start=(j == 0),
                            stop=(j == n - 1),
                        )
                    # softmax row-sum rides on the same stationary operand
                    nc.tensor.matmul(
                        lsums[:, s : s + 1],
                        lhsT=blk,
                        rhs=ones_bf[:],
                        start=(j == 0),
                        stop=(j == n - 1),
                    )
                rl = small_pool.tile([P, 1], F32, name="rl", tag="rl")
                nc.vector.reciprocal(rl[:], lsums[:, s : s + 1])
                rt = rt_pool.tile([P, 1024], BF16, name="rt", tag="rt")
                for h in range(2):
                    nc.vector.tensor_scalar_mul(
                        rt[:, h * 512 : (h + 1) * 512], rp[:, h * 512 : (h + 1) * 512], rl[:]
                    )
                # R^T -> R chunks [128d, 128q] via xbar DMA, issued here so
                # the latency is covered before tail_b consumes r2
                r2 = r2_pool.tile([P, DC, P], BF16, name="r2", tag="r2")
                nc.sync.dma_start_transpose(r2[:, :, :], rt[:])
                state[s] = r2

            def tail_b(s):
                r2 = state.pop(s)
                ops = [
                    o_ps.tile([P, 512], F32, name="o_t", tag="o_t"),
                    o_ps.tile([P, 512], F32, name="o_t", tag="o_t"),
                ]
                ot = o_pool.tile([P, D], F32, name="ot", tag="ot")
                nq = 2 if s == 7 else 1  # finer final chunks shorten the drain
                for h in range(2):
                    for dc in range(DC):
                        nc.tensor.matmul(
                            ops[h][:],
                            lhsT=r2[:, dc, :],
                            rhs=wv_all[:, dc, h * 512 : (h + 1) * 512],
                            start=(dc == 0),
                            stop=(dc == DC - 1),
                        )
                    for q in range(nq):
                        c0 = h * 512 + q * (512 // nq)
                        cw = 512 // nq
                        nc.scalar.copy(ot[:, c0 : c0 + cw], ops[h][:, q * cw : (q + 1) * cw])
                        nc.scalar.dma_start(
                            out_d[s * P : (s + 1) * P, c0 : c0 + cw],
                            ot[:, c0 : c0 + cw],
                        )

            # software pipeline: tail_a lags its enabling strip by one,
            # tail_b by two, so exp -> R and rp -> rt -> r2-DMA -> Wv apply
            # are covered by neighbouring strips' PE work.
            for j in range(NT):
                strip(j)
                if 8 <= j + 2 < NT:
                    nc.sync.dma_start_transpose(
                        XT_alt[j + 2][:, :, :], XK_g[(j + 2) // 4][:, (j + 2) % 4, :]
                    )
                if j >= 2 and j % 2 == 0:
                    tail_a(j // 2 - 1)
                if j >= 4 and j % 2 == 0:
                    tail_b(j // 2 - 2)
            tail_a(7)
            tail_b(6)
            tail_b(7)


def _get_nc():
    if "nc" not in _compiled:
        _compiled["nc"] = _build_v2()
    return _compiled["nc"]


def _host_mcf(tiles):
    """Per-strip mask coefficients: msk_t[j] = TRI*av_j + s2_j.
    Diagonal strip (tiles[j//2]==j): av=1,s2=0; waste strip
    (tiles[j//2]<j): av=0,s2=-1920; else zero mask."""
    m = np.zeros((P, 32), np.float32)
    for j in range(NT):
        t0 = tiles[j // 2]
        if t0 == j:
            m[:, j] = 1.0
        elif t0 < j:
            m[:, NT + j] = MASK_RAW
    return np.ascontiguousarray(m)


def _pm(a, nt):
    """[nt*128, X] row-major -> partition-major [128, nt, X] contiguous."""
    return np.ascontiguousarray(a.reshape(nt, P, -1).transpose(1, 0, 2))


def kernel(x, Wq, Wk, Wv):
    x = np.ascontiguousarray(np.asarray(x, dtype=np.float32))
    Wq = np.ascontiguousarray(np.asarray(Wq, dtype=np.float32))
    Wk = np.ascontiguousarray(np.asarray(Wk, dtype=np.float32))
    Wv = np.ascontiguousarray(np.asarray(Wv, dtype=np.float32))

    nc = _get_nc()
    bf = ml_dtypes.bfloat16

    # weight fusion: Q and K projections collapse into one matrix
    Wqk_c = _pm((Wq @ Wk.T).astype(bf), DC)
    Wv_c = _pm(Wv.astype(bf), DC)
    in_maps = []
    for c in range(8):
        b, par = c // 2, c % 2
        tiles = A_TILES if par == 0 else B_TILES
        xb = x[b]
        xq = np.concatenate([xb[t * P : (t + 1) * P] for t in tiles], axis=0)
        xqT_pm = _pm(xq.T.astype(bf), DC)  # [128, 8, 1024]
        im = {
            "xqh0": np.ascontiguousarray(xqT_pm[:, :, 0:512]),
            "xqh1": np.ascontiguousarray(xqT_pm[:, :, 512:1024]),
            "xk": _pm(xb.astype(bf), NT),
            "wqk": Wqk_c,
            "wv": Wv_c,
            "mcf": _host_mcf(tiles),
        }
        in_maps.append(im)

    trace = os.environ.get("BASS_KERNEL_TRACE", "0") == "1"
    res = run_bass_kernel_spmd(nc, in_maps, core_ids=list(range(8)), trace=trace)
    if trace:
        print(f"HW exec time: {res.exec_time_ns} ns")
        if res.instructions_and_trace is not None:
            print(f"trace: {res.instructions_and_trace[1]}")

    out = np.empty((B, S, D), np.float32)
    for c in range(8):
        b, par = c // 2, c % 2
        tiles = A_TILES if par == 0 else B_TILES
        o = res.results[c]["out"]
        for s_i, t in enumerate(tiles):
            out[b, t * P : (t + 1) * P] = o[s_i * P : (s_i + 1) * P]
    return out


# revision 15
# speedup vs baseline: 1.0670x; 1.0670x over previous
"""Causal self-attention (B=4, S=2048, D=1024, single head) on 8 TRN2 cores.

Sharding: data-parallel over batch (4 batches x 2 cores). The two cores of a
batch split the 16 query tiles (128 rows each): core A takes tiles
{0,2,4,6,9,11,13,15}, core B the complement - slot s (s=0..7) of either core
attends to strips j=0..2s+1 (2s+2 key tiles), so both cores do exactly 72
key-tile units of causal work under ONE shared SPMD program (the <=1 waste
tile per slot is killed by the mask input).

Dataflow (v6): the Q and K projections are fused on the host into
W_qk = Wq @ Wk^T (weights-only preprocessing), so the device computes
Q'' = x_q @ W_qk in ONE projection and scores = Q'' X^T directly.  Scores
are computed TRANSPOSED, key-tile-outer: strip j = [128 keys x all queries
of slots >= j//2], so the exp'd strips (pbT) come out of the PE already
key-major and the P^T transposes of the old dataflow vanish.  The causal
mask is a single [128,128] DVE add on each strip's first query block
(host-built 0/-1920/upper-tri table at raw-score scale; exp applies the
1/32 softmax scale via the ACT scale field).  Per slot: R^T = P X
accumulates over the slot's pbT strips with the softmax row-sum riding
along as an N=1 ones-matmul on the same stationary operand; R^T is
normalized by 1/rowsum during the PSUM->SBUF copy, block-transposed to
d-major via one xbar DMA, and multiplied by Wv.  Strips and slot tails are
software-pipelined (tail_a lags its enabling strip by one, tail_b by two).

DMA: all inputs are partition-major ([128, ...], one contiguous run per
partition -> ~128 descriptors/trigger; the trigger queue costs ~5ns per
descriptor, so layout determines whether the input stream keeps up), issued
in consumer-deadline order; XT ships as 8 per-dc tiles and XK as 4
j-quarter tiles so consumers start in arrival order.  All matmuls bf16:
fp8 DoubleRow was simulated end-to-end and fails the 2e-2 gate (each fp8
operand alone contributes ~2.5-3e-2 because sharp-attention rows carry
~10x magnitude).  PE work/core 4.56G MAC ~= 116us at 2.4GHz; measured
145-150us wall (rel err 3.2e-3) vs 167-169us baseline.  Run-to-run
variance is +-2us warm, +20us when the part drops to 2.0GHz (P0) under
sustained load.
"""

import os
from contextlib import ExitStack

import ml_dtypes
import numpy as np

import concourse.bacc as bacc
import concourse.mybir as mybir
import concourse.tile as tile
from concourse.bass_utils import run_bass_kernel_spmd

B, S, D = 4, 2048, 1024
P = 128
DC = D // P  # 8 contraction chunks
NT = S // P  # 16 key tiles
A_TILES = [0, 2, 4, 6, 9, 11, 13, 15]
B_TILES = [1, 3, 5, 7, 8, 10, 12, 14]
MASK_RAW = -1920.0  # -60 at logit scale, applied pre-ACT at raw score scale

F32 = mybir.dt.float32
BF16 = mybir.dt.bfloat16

_compiled = {}


def _build_v2():
    nc = bacc.Bacc("TRN2", target_bir_lowering=False, debug=False)
    # all inputs partition-major ([128, ...] with one contiguous run per
    # partition) so every DMA trigger is ~128 descriptors (~0.7us issue)
    xqh0 = nc.dram_tensor("xqh0", [P, DC, 512], BF16, kind="ExternalInput").ap()
    xqh1 = nc.dram_tensor("xqh1", [P, DC, 512], BF16, kind="ExternalInput").ap()
    xT = nc.dram_tensor("xT", [P, DC, S], BF16, kind="ExternalInput").ap()
    xk = nc.dram_tensor("xk", [P, NT, D], BF16, kind="ExternalInput").ap()
    wqk = nc.dram_tensor("wqk", [P, DC, D], BF16, kind="ExternalInput").ap()
    wv = nc.dram_tensor("wv", [P, DC, D], BF16, kind="ExternalInput").ap()
    msk = nc.dram_tensor("msk", [P, NT, P], BF16, kind="ExternalInput").ap()
    out_d = nc.dram_tensor("out", [1024, D], F32, kind="ExternalOutput").ap()

    with tile.TileContext(nc) as tc:
        _body_v2(tc, xqh0, xqh1, xT, xk, wqk, wv, msk, out_d)
    nc.compile()
    return nc


def _body_v2(tc, xqh0, xqh1, xT, xk, wqk, wv, msk, out_d):
    nc = tc.nc
    with ExitStack() as top:
        const_pool = top.enter_context(tc.tile_pool(name="cst", bufs=1))
        ones_bf = const_pool.tile([P, 1], BF16, name="ones_bf", tag="ones")
        nc.gpsimd.memset(ones_bf[:], 1.0)

        # whole-kernel residents (single big tiles -> few DMA triggers; the
        # DMA trigger queue issues one DIRECT2D per ~600ns, so 72 separate
        # dma_starts would gate the strip phase by ~9us)
        res_pool = top.enter_context(tc.tile_pool(name="res", bufs=1))
        # per-dc XT tiles and per-quarter XK tiles: separate tiles give
        # separate DMA-completion deps, so consumers start in arrival order
        XT_t = [res_pool.tile([P, S], BF16, name=f"xt{d}", tag=f"xt{d}") for d in range(DC)]
        XK_g = [
            res_pool.tile([P, 4, D], BF16, name=f"xkg{g}", tag=f"xkg{g}") for g in range(4)
        ]
        Q2T = [res_pool.tile([P, 1024], BF16, name=f"q2t{d}", tag=f"q2t{d}") for d in range(DC)]
        wv_all = res_pool.tile([P, DC, D], BF16, name="wv_all", tag="wv_all")
        pbT = [res_pool.tile([P, 1024], BF16, name=f"pbt{j}", tag=f"pbt{j}") for j in range(NT)]
        msk_all = res_pool.tile([P, NT, P], BF16, name="msk_all", tag="msk_all")

        # ---------------- Q'' projection (Q''^T = W_qk^T xq^T) ----------------
        with ExitStack() as ph:
            wqk_pool = ph.enter_context(tc.tile_pool(name="wqkp", bufs=1))
            xq_pool = ph.enter_context(tc.tile_pool(name="xqp", bufs=1))
            pps = ph.enter_context(tc.tile_pool(name="pps", bufs=1, space="PSUM"))

            wqk_all = wqk_pool.tile([P, DC, D], BF16, name="wqk_all", tag="wqk_all")
            xq_h = [
                xq_pool.tile([P, DC, 512], BF16, name=f"xq_h{h}", tag=f"xq_h{h}")
                for h in range(2)
            ]

            # DMA issue order = priority, matched to consumer deadlines:
            # phase-1 strip pairs paced per-e, then xq half 1 (qc=1 sweeps),
            # XT+first masks (strip 0 at ~40us), XK in j-quarters (tail_a
            # deadlines), wv (first tail_b ~58us), rest.
            for e in range(DC):
                nc.sync.dma_start(wqk_all[:, e, :], wqk[:, e, :])
                nc.sync.dma_start(xq_h[0][:, e, :], xqh0[:, e, :])
            nc.sync.dma_start(xq_h[1][:, 0:4, :], xqh1[:, 0:4, :])
            nc.sync.dma_start(xq_h[1][:, 4:DC, :], xqh1[:, 4:DC, :])
            for d in range(DC):
                nc.sync.dma_start(XT_t[d][:], xT[:, d, :])
            nc.sync.dma_start(msk_all[:, :, :], msk[:, :, :])
            nc.sync.dma_start(XK_g[0][:, :, :], xk[:, 0:4, :])
            nc.sync.dma_start(wv_all[:, :, :], wv[:, :, :])
            nc.sync.dma_start(XK_g[1][:, :, :], xk[:, 4:8, :])
            nc.sync.dma_start(XK_g[2][:, :, :], xk[:, 8:12, :])
            nc.sync.dma_start(XK_g[3][:, :, :], xk[:, 12:NT, :])

            # e-outer accumulation across 8 live PSUM banks: the e=0 sweep
            # starts as soon as the first strip pair lands.
            for qc in range(2):
                qps = [pps.tile([P, 512], F32, name="pps", tag=f"pps{dc}") for dc in range(DC)]
                for e in range(DC):
                    for dc in range(DC):
                        nc.tensor.matmul(
                            qps[dc][:],
                            lhsT=wqk_all[:, e, dc * P : (dc + 1) * P],
                            rhs=xq_h[qc][:, e, :],
                            start=(e == 0),
                            stop=(e == DC - 1),
                        )
                for dc in range(DC):
                    nc.scalar.copy(Q2T[dc][:, qc * 512 : (qc + 1) * 512], qps[dc][:])

        # ---------------- attention: key-tile-outer strips + slot tails ----
        with ExitStack() as ph:
            rt_pool = ph.enter_context(tc.tile_pool(name="rtp", bufs=2))
            r2_pool = ph.enter_context(tc.tile_pool(name="r2p", bufs=2))
            o_pool = ph.enter_context(tc.tile_pool(name="op", bufs=2))
            small_pool = ph.enter_context(tc.tile_pool(name="smallp", bufs=3))
            qk_ps = ph.enter_context(tc.tile_pool(name="qkps", bufs=3, space="PSUM"))
            rp_ps = ph.enter_context(tc.tile_pool(name="rps", bufs=1, space="PSUM"))
            o_ps = ph.enter_context(tc.tile_pool(name="ops", bufs=2, space="PSUM"))
            ls_ps = ph.enter_context(tc.tile_pool(name="lsps", bufs=1, space="PSUM"))

            lsums = ls_ps.tile([P, 8], F32, name="lsums", tag="lsums")
            state = {}

            def strip(j):
                qlo = (j // 2) * P
                w = 1024 - qlo
                for c0 in range(qlo, 1024, 512):
                    cw = min(512, 1024 - c0)
                    ps = qk_ps.tile([P, 512], F32, name="qk_t", tag="qk_t")
                    for dc in range(DC):
                        nc.tensor.matmul(
                            ps[:, :cw],
                            lhsT=XT_t[dc][:, j * P : (j + 1) * P],
                            rhs=Q2T[dc][:, c0 : c0 + cw],
                            start=(dc == 0),
                            stop=(dc == DC - 1),
                        )
                    if c0 == qlo:
                        # causal mask lands only on the strip's first query
                        # block (host-built: 0 / -1920 / upper-tri)
                        nc.vector.tensor_add(ps[:, 0:P], ps[:, 0:P], msk_all[:, j, :])
                    nc.scalar.activation(
                        pbT[j][:, c0 : c0 + cw],
                        ps[:, :cw],
                        mybir.ActivationFunctionType.Exp,
                        scale=1.0 / 32.0,
                    )

            def tail_a(s):
                n = 2 * s + 2  # strips for this slot
                rp = rp_ps.tile([P, 1024], F32, name="rp", tag="rp")
                for j in range(n):
                    blk = pbT[j][:, s * P : (s + 1) * P]
                    for h in range(2):
                        nc.tensor.matmul(
                            rp[:, h * 512 : (h + 1) * 512],
                            lhsT=blk,
                            rhs=XK_g[j // 4][:, j % 4, h * 512 : (h + 1) * 512],
                            start=(j == 0),
                            stop=(j == n - 1),
                        )
                    # softmax row-sum rides on the same stationary operand
                    nc.tensor.matmul(
                        lsums[:, s : s + 1],
                        lhsT=blk,
                        rhs=ones_bf[:],
                        start=(j == 0),
                        stop=(j == n - 1),
                    )
                rl = small_pool.tile([P, 1], F32, name="rl", tag="rl")
                nc.vector.reciprocal(rl[:], lsums[:, s : s + 1])
                rt = rt_pool.tile([P, 1024], BF16, name="rt", tag="rt")
                for h in range(2):
                    nc.vector.tensor_scalar_mul(
                        rt[:, h * 512 : (h + 1) * 512], rp[:, h * 512 : (h + 1) * 512], rl[:]
                    )
                # R^T -> R chunks [128d, 128q] via xbar DMA, issued here so
                # the latency is covered before tail_b consumes r2
                r2 = r2_pool.tile([P, DC, P], BF16, name="r2", tag="r2")
                nc.sync.dma_start_transpose(r2[:, :, :], rt[:])
                state[s] = r2

            def tail_b(s):
                r2 = state.pop(s)
                ops = [
                    o_ps.tile([P, 512], F32, name="o_t", tag="o_t"),
                    o_ps.tile([P, 512], F32, name="o_t", tag="o_t"),
                ]
                ot = o_pool.tile([P, D], F32, name="ot", tag="ot")
                nq = 2 if s == 7 else 1  # finer final chunks shorten the drain
                for h in range(2):
                    for dc in range(DC):
                        nc.tensor.matmul(
                            ops[h][:],
                            lhsT=r2[:, dc, :],
                            rhs=wv_all[:, dc, h * 512 : (h + 1) * 512],
                            start=(dc == 0),
                            stop=(dc == DC - 1),
                        )
                    for q in range(nq):
                        c0 = h * 512 + q * (512 // nq)
                        cw = 512 // nq
                        nc.scalar.copy(ot[:, c0 : c0 + cw], ops[h][:, q * cw : (q + 1) * cw])
                        nc.scalar.dma_start(
                            out_d[s * P : (s + 1) * P, c0 : c0 + cw],
                            ot[:, c0 : c0 + cw],
                        )

            # software pipeline: tail_a lags its enabling strip by one,
            # tail_b by two, so exp -> R and rp -> rt -> r2-DMA -> Wv apply
            # are covered by neighbouring strips' PE work.
            for j in range(NT):
                strip(j)
                if j >= 2 and j % 2 == 0:
                    tail_a(j // 2 - 1)
                if j >= 4 and j % 2 == 0:
                    tail_b(j // 2 - 2)
            tail_a(7)
            tail_b(6)
            tail_b(7)


def _get_nc():
    if "nc" not in _compiled:
        _compiled["nc"] = _build_v2()
    return _compiled["nc"]


def _host_mask(tiles):
    """msk[p, j, c] = 0 if key (j*128+p) <= query (tiles[j//2]*128+c)
    else -1920 (raw-score scale); partition-major."""
    m = np.zeros((P, NT, P), np.float32)
    karr = np.arange(P)
    for j in range(NT):
        t0 = tiles[j // 2]
        keys = j * P + karr
        qrows = t0 * P + karr
        m[:, j, :] = np.where(
            keys[:, None] <= qrows[None, :], np.float32(0.0), np.float32(MASK_RAW)
        )
    return m


def _pm(a, nt):
    """[nt*128, X] row-major -> partition-major [128, nt, X] contiguous."""
    return np.ascontiguousarray(a.reshape(nt, P, -1).transpose(1, 0, 2))


def kernel(x, Wq, Wk, Wv):
    x = np.ascontiguousarray(np.asarray(x, dtype=np.float32))
    Wq = np.ascontiguousarray(np.asarray(Wq, dtype=np.float32))
    Wk = np.ascontiguousarray(np.asarray(Wk, dtype=np.float32))
    Wv = np.ascontiguousarray(np.asarray(Wv, dtype=np.float32))

    nc = _get_nc()
    bf = ml_dtypes.bfloat16

    # weight fusion: Q and K projections collapse into one matrix
    Wqk_c = _pm((Wq @ Wk.T).astype(bf), DC)
    Wv_c = _pm(Wv.astype(bf), DC)
    in_maps = []
    for c in range(8):
        b, par = c // 2, c % 2
        tiles = A_TILES if par == 0 else B_TILES
        xb = x[b]
        xq = np.concatenate([xb[t * P : (t + 1) * P] for t in tiles], axis=0)
        xqT_pm = _pm(xq.T.astype(bf), DC)  # [128, 8, 1024]
        im = {
            "xqh0": np.ascontiguousarray(xqT_pm[:, :, 0:512]),
            "xqh1": np.ascontiguousarray(xqT_pm[:, :, 512:1024]),
            "xT": _pm(xb.T.astype(bf), DC),
            "xk": _pm(xb.astype(bf), NT),
            "wqk": Wqk_c,
            "wv": Wv_c,
            "msk": np.ascontiguousarray(_host_mask(tiles).astype(bf)),
        }
        in_maps.append(im)

    trace = os.environ.get("BASS_KERNEL_TRACE", "0") == "1"
    res = run_bass_kernel_spmd(nc, in_maps, core_ids=list(range(8)), trace=trace)
    if trace:
        print(f"HW exec time: {res.exec_time_ns} ns")
        if res.instructions_and_trace is not None:
            print(f"trace: {res.instructions_and_trace[1]}")

    out = np.empty((B, S, D), np.float32)
    for c in range(8):
        b, par = c // 2, c % 2
        tiles = A_TILES if par == 0 else B_TILES
        o = res.results[c]["out"]
        for s_i, t in enumerate(tiles):
            out[b, t * P : (t + 1) * P] = o[s_i * P : (s_i + 1) * P]
    return out


# revision 16
# speedup vs baseline: 1.0687x; 1.0017x over previous
"""Causal self-attention (B=4, S=2048, D=1024, single head) on 8 TRN2 cores.

Sharding: data-parallel over batch (4 batches x 2 cores). The two cores of a
batch split the 16 query tiles (128 rows each): core A takes tiles
{0,2,4,6,9,11,13,15}, core B the complement - slot s (s=0..7) of either core
attends to strips j=0..2s+1 (2s+2 key tiles), so both cores do exactly 72
key-tile units of causal work under ONE shared SPMD program (the <=1 waste
tile per slot is killed by the mask input).

Dataflow: the Q and K projections are fused on the host into
W_qk = Wq @ Wk^T (weights-only preprocessing), so the device computes
Q'' = x_q @ W_qk in ONE projection and scores = Q'' X^T directly.  Scores
are computed TRANSPOSED, key-tile-outer: strip j = [128 keys x all queries
of slots >= j//2], so the exp'd strips (pbT) come out of the PE already
key-major and the P^T transposes of the old dataflow vanish.  The causal
mask is a single [128,128] DVE add on each strip's first query block
(host-built 0/-1920/upper-tri table at raw-score scale; exp applies the
1/32 softmax scale via the ACT scale field).  Per slot: R^T = P X
accumulates over the slot's pbT strips with the softmax row-sum riding
along as an N=1 ones-matmul on the same stationary operand; R^T is
normalized by 1/rowsum during the PSUM->SBUF copy, block-transposed to
d-major via one xbar DMA, and multiplied by Wv.  Strips and slot tails are
software-pipelined (tail_a lags its enabling strip by one, tail_b by two).

DMA: all inputs are partition-major ([128, ...], one contiguous run per
partition -> ~128 descriptors/trigger; the trigger queue costs ~5ns per
descriptor, so layout determines whether the input stream keeps up), issued
in consumer-deadline order; XT ships as 8 per-dc tiles and XK as 4
j-quarter tiles so consumers start in arrival order.  All matmuls bf16:
fp8 DoubleRow was simulated end-to-end and fails the 2e-2 gate (each fp8
operand alone contributes ~2.5-3e-2 because sharp-attention rows carry
~10x magnitude).  PE work/core 4.56G MAC ~= 116us at 2.4GHz; measured
144.9-149us wall (rel err 3.2e-3) vs 167-169us baseline.  Run-to-run
variance is +-2us warm, +20us when the part drops to 2.0GHz (P0) under
sustained load.  Tested-and-rejected variants (interleaved A/B medians):
on-device xbar transposes for XT (+3.5us), device-built masks + split
first trigger (+0.8us), output copies on DVE instead of ScalarE (+2us).
"""

import os
from contextlib import ExitStack

import ml_dtypes
import numpy as np

import concourse.bacc as bacc
import concourse.mybir as mybir
import concourse.tile as tile
from concourse.bass_utils import run_bass_kernel_spmd

B, S, D = 4, 2048, 1024
P = 128
DC = D // P  # 8 contraction chunks
NT = S // P  # 16 key tiles
A_TILES = [0, 2, 4, 6, 9, 11, 13, 15]
B_TILES = [1, 3, 5, 7, 8, 10, 12, 14]
MASK_RAW = -1920.0  # -60 at logit scale, applied pre-ACT at raw score scale

F32 = mybir.dt.float32
BF16 = mybir.dt.bfloat16

_compiled = {}


def _build_v2():
    nc = bacc.Bacc("TRN2", target_bir_lowering=False, debug=False)
    # all inputs partition-major ([128, ...] with one contiguous run per
    # partition) so every DMA trigger is ~128 descriptors (~0.7us issue)
    xqh0 = nc.dram_tensor("xqh0", [P, DC, 512], BF16, kind="ExternalInput").ap()
    xqh1 = nc.dram_tensor("xqh1", [P, DC, 512], BF16, kind="ExternalInput").ap()
    xT = nc.dram_tensor("xT", [P, DC, S], BF16, kind="ExternalInput").ap()
    xk = nc.dram_tensor("xk", [P, NT, D], BF16, kind="ExternalInput").ap()
    wqk = nc.dram_tensor("wqk", [P, DC, D], BF16, kind="ExternalInput").ap()
    wv = nc.dram_tensor("wv", [P, DC, D], BF16, kind="ExternalInput").ap()
    msk = nc.dram_tensor("msk", [P, NT, P], BF16, kind="ExternalInput").ap()
    out_d = nc.dram_tensor("out", [1024, D], F32, kind="ExternalOutput").ap()

    with tile.TileContext(nc) as tc:
        _body_v2(tc, xqh0, xqh1, xT, xk, wqk, wv, msk, out_d)
    nc.compile()
    return nc


def _body_v2(tc, xqh0, xqh1, xT, xk, wqk, wv, msk, out_d):
    nc = tc.nc
    with ExitStack() as top:
        const_pool = top.enter_context(tc.tile_pool(name="cst", bufs=1))
        ones_bf = const_pool.tile([P, 1], BF16, name="ones_bf", tag="ones")
        nc.gpsimd.memset(ones_bf[:], 1.0)

        # whole-kernel residents (single big tiles -> few DMA triggers; the
        # DMA trigger queue issues one DIRECT2D per ~600ns, so 72 separate
        # dma_starts would gate the strip phase by ~9us)
        res_pool = top.enter_context(tc.tile_pool(name="res", bufs=1))
        # per-dc XT tiles and per-quarter XK tiles: separate tiles give
        # separate DMA-completion deps, so consumers start in arrival order
        XT_t = [res_pool.tile([P, S], BF16, name=f"xt{d}", tag=f"xt{d}") for d in range(DC)]
        XK_g = [
            res_pool.tile([P, 4, D], BF16, name=f"xkg{g}", tag=f"xkg{g}") for g in range(4)
        ]
        Q2T = [res_pool.tile([P, 1024], BF16, name=f"q2t{d}", tag=f"q2t{d}") for d in range(DC)]
        wv_all = res_pool.tile([P, DC, D], BF16, name="wv_all", tag="wv_all")
        pbT = [res_pool.tile([P, 1024], BF16, name=f"pbt{j}", tag=f"pbt{j}") for j in range(NT)]
        msk_all = res_pool.tile([P, NT, P], BF16, name="msk_all", tag="msk_all")

        # ---------------- Q'' projection (Q''^T = W_qk^T xq^T) ----------------
        with ExitStack() as ph:
            wqk_pool = ph.enter_context(tc.tile_pool(name="wqkp", bufs=1))
            xq_pool = ph.enter_context(tc.tile_pool(name="xqp", bufs=1))
            pps = ph.enter_context(tc.tile_pool(name="pps", bufs=1, space="PSUM"))

            wqk_all = wqk_pool.tile([P, DC, D], BF16, name="wqk_all", tag="wqk_all")
            xq_h = [
                xq_pool.tile([P, DC, 512], BF16, name=f"xq_h{h}", tag=f"xq_h{h}")
                for h in range(2)
            ]

            # DMA issue order = priority, matched to consumer deadlines:
            # phase-1 strip pairs paced per-e, then xq half 1 (qc=1 sweeps),
            # XT+first masks (strip 0 at ~40us), XK in j-quarters (tail_a
            # deadlines), wv (first tail_b ~58us), rest.
            for e in range(DC):
                nc.sync.dma_start(wqk_all[:, e, :], wqk[:, e, :])
                nc.sync.dma_start(xq_h[0][:, e, :], xqh0[:, e, :])
            nc.sync.dma_start(xq_h[1][:, 0:4, :], xqh1[:, 0:4, :])
            nc.sync.dma_start(xq_h[1][:, 4:DC, :], xqh1[:, 4:DC, :])
            for d in range(DC):
                nc.sync.dma_start(XT_t[d][:], xT[:, d, :])
            nc.sync.dma_start(msk_all[:, :, :], msk[:, :, :])
            nc.sync.dma_start(XK_g[0][:, :, :], xk[:, 0:4, :])
            nc.sync.dma_start(wv_all[:, :, :], wv[:, :, :])
            nc.sync.dma_start(XK_g[1][:, :, :], xk[:, 4:8, :])
            nc.sync.dma_start(XK_g[2][:, :, :], xk[:, 8:12, :])
            nc.sync.dma_start(XK_g[3][:, :, :], xk[:, 12:NT, :])

            # e-outer accumulation across 8 live PSUM banks: the e=0 sweep
            # starts as soon as the first strip pair lands.
            for qc in range(2):
                qps = [pps.tile([P, 512], F32, name="pps", tag=f"pps{dc}") for dc in range(DC)]
                for e in range(DC):
                    for dc in range(DC):
                        nc.tensor.matmul(
                            qps[dc][:],
                            lhsT=wqk_all[:, e, dc * P : (dc + 1) * P],
                            rhs=xq_h[qc][:, e, :],
                            start=(e == 0),
                            stop=(e == DC - 1),
                        )
                for dc in range(DC):
                    nc.scalar.copy(Q2T[dc][:, qc * 512 : (qc + 1) * 512], qps[dc][:])

        # ---------------- attention: key-tile-outer strips + slot tails ----
        with ExitStack() as ph:
            rt_pool = ph.enter_context(tc.tile_pool(name="rtp", bufs=2))
            r2_pool = ph.enter_context(tc.tile_pool(name="r2p", bufs=2))
            o_pool = ph.enter_context(tc.tile_pool(name="op", bufs=2))
            small_pool = ph.enter_context(tc.tile_pool(name="smallp", bufs=3))
            qk_ps = ph.enter_context(tc.tile_pool(name="qkps", bufs=3, space="PSUM"))
            rp_ps = ph.enter_context(tc.tile_pool(name="rps", bufs=1, space="PSUM"))
            o_ps = ph.enter_context(tc.tile_pool(name="ops", bufs=2, space="PSUM"))
            ls_ps = ph.enter_context(tc.tile_pool(name="lsps", bufs=1, space="PSUM"))

            lsums = ls_ps.tile([P, 8], F32, name="lsums", tag="lsums")
            state = {}

            def strip(j):
                qlo = (j // 2) * P
                w = 1024 - qlo
                for c0 in range(qlo, 1024, 512):
                    cw = min(512, 1024 - c0)
                    ps = qk_ps.tile([P, 512], F32, name="qk_t", tag="qk_t")
                    for dc in range(DC):
                        nc.tensor.matmul(
                            ps[:, :cw],
                            lhsT=XT_t[dc][:, j * P : (j + 1) * P],
                            rhs=Q2T[dc][:, c0 : c0 + cw],
                            start=(dc == 0),
                            stop=(dc == DC - 1),
                        )
                    if c0 == qlo:
                        # causal mask lands only on the strip's first query
                        # block (host-built: 0 / -1920 / upper-tri)
                        nc.vector.tensor_add(ps[:, 0:P], ps[:, 0:P], msk_all[:, j, :])
                    nc.scalar.activation(
                        pbT[j][:, c0 : c0 + cw],
                        ps[:, :cw],
                        mybir.ActivationFunctionType.Exp,
                        scale=1.0 / 32.0,
                    )

            def tail_a(s):
                n = 2 * s + 2  # strips for this slot
                rp = rp_ps.tile([P, 1024], F32, name="rp", tag="rp")
                for j in range(n):
                    blk = pbT[j][:, s * P : (s + 1) * P]
                    for h in range(2):
                        nc.tensor.matmul(
                            rp[:, h * 512 : (h + 1) * 512],
                            lhsT=blk,
                            rhs=XK_g[j // 4][:, j % 4, h * 512 : (h + 1) * 512],
                            start=(j == 0),
                            stop=(j == n - 1),
                        )
                    # softmax row-sum rides on the same stationary operand
                    nc.tensor.matmul(
                        lsums[:, s : s + 1],
                        lhsT=blk,
                        rhs=ones_bf[:],
                        start=(j == 0),
                        stop=(j == n - 1),
                    )
                rl = small_pool.tile([P, 1], F32, name="rl", tag="rl")
                nc.vector.reciprocal(rl[:], lsums[:, s : s + 1])
                rt = rt_pool.tile([P, 1024], BF16, name="rt", tag="rt")
                for h in range(2):
                    nc.vector.tensor_scalar_mul(
                        rt[:, h * 512 : (h + 1) * 512], rp[:, h * 512 : (h + 1) * 512], rl[:]
                    )
                # R^T -> R chunks [128d, 128q] via xbar DMA, issued here so
                # the latency is covered before tail_b consumes r2
                r2 = r2_pool.tile([P, DC, P], BF16, name="r2", tag="r2")
                nc.sync.dma_start_transpose(r2[:, :, :], rt[:])
                state[s] = r2

            def tail_b(s):
                r2 = state.pop(s)
                ops = [
                    o_ps.tile([P, 512], F32, name="o_t", tag="o_t"),
                    o_ps.tile([P, 512], F32, name="o_t", tag="o_t"),
                ]
                ot = o_pool.tile([P, D], F32, name="ot", tag="ot")
                nq = 2 if s == 7 else 1  # finer final chunks shorten the drain
                for h in range(2):
                    for dc in range(DC):
                        nc.tensor.matmul(
                            ops[h][:],
                            lhsT=r2[:, dc, :],
                            rhs=wv_all[:, dc, h * 512 : (h + 1) * 512],
                            start=(dc == 0),
                            stop=(dc == DC - 1),
                        )
                    for q in range(nq):
                        c0 = h * 512 + q * (512 // nq)
                        cw = 512 // nq
                        nc.scalar.copy(ot[:, c0 : c0 + cw], ops[h][:, q * cw : (q + 1) * cw])
                        nc.scalar.dma_start(
                            out_d[s * P : (s + 1) * P, c0 : c0 + cw],
                            ot[:, c0 : c0 + cw],
                        )

            # software pipeline: tail_a lags its enabling strip by one,
            # tail_b by two, so exp -> R and rp -> rt -> r2-DMA -> Wv apply
            # are covered by neighbouring strips' PE work.
            for j in range(NT):
                strip(j)
                if j >= 2 and j % 2 == 0:
                    tail_a(j // 2 - 1)
                if j >= 4 and j % 2 == 0:
                    tail_b(j // 2 - 2)
            tail_a(7)
            tail_b(6)
            tail_b(7)


def _get_nc():
    if "nc" not in _compiled:
        _compiled["nc"] = _build_v2()
    return _compiled["nc"]


def _host_mask(tiles):
    """msk[p, j, c] = 0 if key (j*128+p) <= query (tiles[j//2]*128+c)
    else -1920 (raw-score scale); partition-major."""
    m = np.zeros((P, NT, P), np.float32)
    karr = np.arange(P)
    for j in range(NT):
        t0 = tiles[j // 2]
        keys = j * P + karr
        qrows = t0 * P + karr
        m[:, j, :] = np.where(
            keys[:, None] <= qrows[None, :], np.float32(0.0), np.float32(MASK_RAW)
        )
    return m


def _pm(a, nt):
    """[nt*128, X] row-major -> partition-major [128, nt, X] contiguous."""
    return np.ascontiguousarray(a.reshape(nt, P, -1).transpose(1, 0, 2))


def kernel(x, Wq, Wk, Wv):
    x = np.ascontiguousarray(np.asarray(x, dtype=np.float32))
    Wq = np.ascontiguousarray(np.asarray(Wq, dtype=np.float32))
    Wk = np.ascontiguousarray(np.asarray(Wk, dtype=np.float32))
    Wv = np.ascontiguousarray(np.asarray(Wv, dtype=np.float32))

    nc = _get_nc()
    bf = ml_dtypes.bfloat16

    # weight fusion: Q and K projections collapse into one matrix
    Wqk_c = _pm((Wq @ Wk.T).astype(bf), DC)
    Wv_c = _pm(Wv.astype(bf), DC)
    in_maps = []
    for c in range(8):
        b, par = c // 2, c % 2
        tiles = A_TILES if par == 0 else B_TILES
        xb = x[b]
        xq = np.concatenate([xb[t * P : (t + 1) * P] for t in tiles], axis=0)
        xqT_pm = _pm(xq.T.astype(bf), DC)  # [128, 8, 1024]
        im = {
            "xqh0": np.ascontiguousarray(xqT_pm[:, :, 0:512]),
            "xqh1": np.ascontiguousarray(xqT_pm[:, :, 512:1024]),
            "xT": _pm(xb.T.astype(bf), DC),
            "xk": _pm(xb.astype(bf), NT),
            "wqk": Wqk_c,
            "wv": Wv_c,
            "msk": np.ascontiguousarray(_host_mask(tiles).astype(bf)),
        }
        in_maps.append(im)

    trace = os.environ.get("BASS_KERNEL_TRACE", "0") == "1"
    res = run_bass_kernel_spmd(nc, in_maps, core_ids=list(range(8)), trace=trace)
    if trace:
        print(f"HW exec time: {res.exec_time_ns} ns")
        if res.instructions_and_trace is not None:
            print(f"trace: {res.instructions_and_trace[1]}")

    out = np.empty((B, S, D), np.float32)
    for c in range(8):
        b, par = c // 2, c % 2
        tiles = A_TILES if par == 0 else B_TILES
        o = res.results[c]["out"]
        for s_i, t in enumerate(tiles):
            out[b, t * P : (t + 1) * P] = o[s_i * P : (s_i + 1) * P]
    return out
